# revision 10
# baseline (speedup 1.0000x reference)
"""PointerNet Trainium2 Bass kernel.

kernel(**inputs) takes full unsharded numpy inputs (setup_inputs()) and
returns (logits, pointers) matching reference(). Batch B=256 is sharded
across 8 NeuronCores (32 rows each) with replicated weights; one SPMD
Bass program computes the whole model on-chip.

Per-core layout notes (col = b*Li + l everywhere):
  - enc_proj^T kept SBUF-resident as two [128, 32*Li] chunk tiles.
  - decoder attention: tanh fused with per-(b,chunk) bias on ACT;
    V-reduction via stationary-lhsT matmuls (T' tile as weights, V col
    streams) accumulating scores directly in PSUM in argmax layout.
  - two-stage argmax: DVE max/max_index per transposed-score row, then
    PE transpose + compare/select for the cross-partition reduction with
    exact first-occurrence tie semantics (matches jnp.argmax).
  - pointer gather: gpsimd dma_gather from a precomputed embedding-row
    table in DRAM (idx replication via a small selector matmul).
  - batch processed in two groups of 16 so each group's serial
    argmax/gather/LSTM chain hides under the other group's ACT stream.
"""
import sys
sys.path.insert(0, "/opt/trn_rl_repo")
import numpy as np
import concourse.bass as bass
import concourse.bacc as bacc
import concourse.tile as tile
from concourse import mybir
from concourse.bass_utils import run_bass_kernel_spmd

f32 = mybir.dt.float32
i32 = mybir.dt.int32
i16 = mybir.dt.int16
u32 = mybir.dt.uint32
AF = mybir.ActivationFunctionType
ALU = mybir.AluOpType
AX = mybir.AxisListType

B, C, E, H = 256, 16, 128, 256
LI_FULL, LO_FULL = 512, 128
NCORES = 8
BL = B // NCORES          # 32 local batch
NG = 2                    # batch groups
GB = BL // NG             # 16 per group
PERM = [0, 1, 2, 3, 6, 7, 4, 5]   # gate chunks (i,i,f,f,o,o,g,g)


def _perm_rows(w):
    return np.concatenate([w[128 * c:128 * (c + 1)] for c in PERM], axis=0)


def build_program(Li, Lo, nonzero_enc_b, nonzero_dec_b, num_devices=NCORES):
    ROWS = BL * Li
    JJ = Li // 128
    nc = bacc.Bacc("TRN2", target_bir_lowering=False, debug=False,
                   num_devices=num_devices)

    def din(name, shape, dt=f32):
        return nc.dram_tensor(name, list(shape), dt, kind="ExternalInput").ap()

    inT = din("inT", [C + 1, ROWS])
    wih_e = din("wih_e", [E, 4 * H])
    whh_e = din("whh_e", [H, 4 * H])
    wih_d = din("wih_d", [E, 4 * H])
    whh_d = din("whh_d", [H, 4 * H])
    w1t = din("w1t", [H, H])
    w2t = din("w2t", [H, H])
    vcol = din("vcol", [128, 2])
    embw = din("embw", [C + 1, E])
    x0t = din("x0t", [E, BL])
    selm = din("selm", [GB, 128])
    ident = din("ident", [128, 128])
    jconst = din("jconst", [128, 1])
    bconst = din("bconst", [1, BL])
    if nonzero_enc_b:
        encb = din("encb", [128, 8 * BL])
    if nonzero_dec_b:
        decb = din("decb", [128, 8 * BL])

    emb_rows = nc.dram_tensor("emb_rows", [ROWS, E], f32).ap()
    stage2 = nc.dram_tensor("stage2", [Lo, NG * JJ * GB, 128], f32).ap()

    logits_o = nc.dram_tensor("logits", [BL, Li, Lo], f32,
                              kind="ExternalOutput").ap()
    ptr_o = nc.dram_tensor("pointers", [BL, Lo], i32,
                           kind="ExternalOutput").ap()

    gsem = nc.alloc_semaphore("gsem")
    gcnt = [0]

    with tile.TileContext(nc) as tc:
        with tc.tile_pool(name="wpool", bufs=1) as wp:
            W = {}
            def load(name, ap, shape, dt=f32):
                t = wp.tile(list(shape), dt, tag=name)
                nc.sync.dma_start(t[:], ap)
                W[name] = t
            load("wih_e", wih_e[:], [E, 4 * H])
            load("whh_e0", whh_e[0:128, :], [128, 4 * H])
            load("whh_e1", whh_e[128:256, :], [128, 4 * H])
            load("wih_d", wih_d[:], [E, 4 * H])
            load("whh_d0", whh_d[0:128, :], [128, 4 * H])
            load("whh_d1", whh_d[128:256, :], [128, 4 * H])
            load("w1t0", w1t[0:128, :], [128, H])
            load("w1t1", w1t[128:256, :], [128, H])
            load("w2t0", w2t[0:128, :], [128, H])
            load("w2t1", w2t[128:256, :], [128, H])
            load("vcol", vcol[:], [128, 2])
            load("embw", embw[:], [C + 1, E])
            load("x0t", x0t[:], [E, BL])
            load("selm", selm[:], [GB, 128])
            load("ident", ident[:], [128, 128])
            load("jconst", jconst[:], [128, 1])
            load("bconst", bconst[:], [1, BL])
            if nonzero_enc_b:
                load("encb", encb[:], [128, 8 * BL])
            if nonzero_dec_b:
                load("decb", decb[:], [128, 8 * BL])

            encproj = [wp.tile([128, ROWS], f32, tag=f"encproj{c}",
                               name=f"encproj{c}")
                       for c in range(2)]
            ptr_row = wp.tile([1, BL * Lo], f32, tag="ptrrow")
            h_fin = wp.tile([128, 2 * BL], f32, tag="h_fin")
            c_fin = wp.tile([128, 2 * BL], f32, tag="c_fin")

            # ================= PROLOGUE: emb_rows table =================
            with tc.tile_pool(name="pro_sb", bufs=3) as psb, \
                 tc.tile_pool(name="pro_ps", bufs=3, space="PSUM") as pps:
                for ch in range(ROWS // 128):
                    itc = psb.tile([C + 1, 128], f32, tag="itc")
                    nc.sync.dma_start(itc[:], inT[:, 128 * ch:128 * (ch + 1)])
                    er_ps = pps.tile([128, E], f32, tag="erps")
                    nc.tensor.matmul(er_ps[:], itc[:], W["embw"][:],
                                     start=True, stop=True)
                    er_sb = psb.tile([128, E], f32, tag="ersb")
                    nc.vector.tensor_copy(er_sb[:], er_ps[:])
                    nc.sync.dma_start(
                        emb_rows[128 * ch:128 * (ch + 1), :], er_sb[:])

            # ================= ENCODER =================
            with tc.tile_pool(name="enc_sb", bufs=3) as esb, \
                 tc.tile_pool(name="enc_st", bufs=2) as est, \
                 tc.tile_pool(name="enc_ps", bufs=2, space="PSUM") as eps:
                hT = est.tile([128, 2 * BL], f32, tag="hT")
                cT = est.tile([128, 2 * BL], f32, tag="cT")
                nc.vector.memset(hT[:], 0.0)
                nc.vector.memset(cT[:], 0.0)

                for t in range(Li):
                    xs = esb.tile([C + 1, BL], f32, tag="xs")
                    nc.sync.dma_start(
                        xs[:],
                        inT.rearrange("c (b l) -> c l b", l=Li)[:, t, :])
                    xcol_ps = eps.tile([E, BL], f32, tag="xcol")
                    nc.tensor.matmul(xcol_ps[:], W["embw"][:], xs[:],
                                     start=True, stop=True)
                    xcol = esb.tile([E, BL], f32, tag="xcolsb")
                    nc.vector.tensor_copy(xcol[:], xcol_ps[:])

                    G = eps.tile([128, 8 * BL], f32, tag="G")
                    for cg in range(8):
                        nc.tensor.matmul(
                            G[:, BL * cg:BL * (cg + 1)],
                            W["wih_e"][:, 128 * cg:128 * (cg + 1)],
                            xcol[:], start=True, stop=False)
                        for k in range(2):
                            whh = W["whh_e0"] if k == 0 else W["whh_e1"]
                            nc.tensor.matmul(
                                G[:, BL * cg:BL * (cg + 1)],
                                whh[:, 128 * cg:128 * (cg + 1)],
                                hT[:, BL * k:BL * (k + 1)],
                                start=False, stop=(k == 1))
                    if nonzero_enc_b:
                        nc.vector.tensor_tensor(G[:], G[:], W["encb"][:],
                                                op=ALU.add)
                    SG = esb.tile([128, 6 * BL], f32, tag="SG")
                    nc.scalar.activation(SG[:], G[:, 0:6 * BL], AF.Sigmoid)
                    TG = esb.tile([128, 2 * BL], f32, tag="TG")
                    nc.scalar.activation(TG[:], G[:, 6 * BL:8 * BL], AF.Tanh)
                    m1 = esb.tile([128, 2 * BL], f32, tag="m1")
                    nc.vector.tensor_tensor(m1[:], SG[:, 0:2 * BL], TG[:],
                                            op=ALU.mult)
                    m2 = esb.tile([128, 2 * BL], f32, tag="m2")
                    nc.vector.tensor_tensor(m2[:], SG[:, 2 * BL:4 * BL],
                                            cT[:], op=ALU.mult)
                    cT = est.tile([128, 2 * BL], f32, tag="cT")
                    nc.vector.tensor_tensor(cT[:], m1[:], m2[:], op=ALU.add)
                    TC = esb.tile([128, 2 * BL], f32, tag="TC")
                    nc.scalar.activation(TC[:], cT[:], AF.Tanh)
                    hT = est.tile([128, 2 * BL], f32, tag="hT")
                    nc.vector.tensor_tensor(hT[:], SG[:, 4 * BL:6 * BL],
                                            TC[:], op=ALU.mult)

                    pcol = eps.tile([128, 2 * BL], f32, tag="pcol")
                    for cp in range(2):
                        for k in range(2):
                            w1 = W["w1t0"] if k == 0 else W["w1t1"]
                            nc.tensor.matmul(
                                pcol[:, BL * cp:BL * (cp + 1)],
                                w1[:, 128 * cp:128 * (cp + 1)],
                                hT[:, BL * k:BL * (k + 1)],
                                start=(k == 0), stop=(k == 1))
                    for cp in range(2):
                        nc.vector.tensor_copy(
                            encproj[cp].rearrange(
                                "h (b l) -> h l b", l=Li)[:, t, :],
                            pcol[:, BL * cp:BL * (cp + 1)])

                nc.vector.tensor_copy(h_fin[:], hT[:])
                nc.vector.tensor_copy(c_fin[:], cT[:])

            # ================= DECODER =================
            with tc.tile_pool(name="dec_sb", bufs=3) as dsb, \
                 tc.tile_pool(name="dec_tp", bufs=4) as dtp, \
                 tc.tile_pool(name="dec_st", bufs=2) as dst, \
                 tc.tile_pool(name="psG", bufs=2, space="PSUM") as psG, \
                 tc.tile_pool(name="psS", bufs=1, space="PSUM") as psS, \
                 tc.tile_pool(name="psT", bufs=2, space="PSUM") as psT, \
                 tc.tile_pool(name="psD", bufs=2, space="PSUM") as psD:

                hT = h_fin
                cT = c_fin
                xT = W["x0t"]

                for t in range(Lo):
                    G = psG.tile([128, 8 * BL], f32, tag="G2")
                    for cg in range(8):
                        for k in range(2):
                            whh = W["whh_d0"] if k == 0 else W["whh_d1"]
                            nc.tensor.matmul(
                                G[:, BL * cg:BL * (cg + 1)],
                                whh[:, 128 * cg:128 * (cg + 1)],
                                hT[:, BL * k:BL * (k + 1)],
                                start=(k == 0), stop=False)
                        nc.tensor.matmul(
                            G[:, BL * cg:BL * (cg + 1)],
                            W["wih_d"][:, 128 * cg:128 * (cg + 1)],
                            xT[:], start=False, stop=True)
                    if nonzero_dec_b:
                        nc.vector.tensor_tensor(G[:], G[:], W["decb"][:],
                                                op=ALU.add)
                    SG = dsb.tile([128, 6 * BL], f32, tag="SGd")
                    nc.scalar.activation(SG[:], G[:, 0:6 * BL], AF.Sigmoid)
                    TG = dsb.tile([128, 2 * BL], f32, tag="TGd")
                    nc.scalar.activation(TG[:], G[:, 6 * BL:8 * BL], AF.Tanh)
                    m1 = dsb.tile([128, 2 * BL], f32, tag="m1d")
                    nc.vector.tensor_tensor(m1[:], SG[:, 0:2 * BL], TG[:],
                                            op=ALU.mult)
                    m2 = dsb.tile([128, 2 * BL], f32, tag="m2d")
                    nc.vector.tensor_tensor(m2[:], SG[:, 2 * BL:4 * BL],
                                            cT[:], op=ALU.mult)
                    cT = dst.tile([128, 2 * BL], f32, tag="cTd")
                    nc.vector.tensor_tensor(cT[:], m1[:], m2[:], op=ALU.add)
                    TC = dsb.tile([128, 2 * BL], f32, tag="TCd")
                    nc.scalar.activation(TC[:], cT[:], AF.Tanh)
                    hT = dst.tile([128, 2 * BL], f32, tag="hTd")
                    nc.vector.tensor_tensor(hT[:], SG[:, 4 * BL:6 * BL],
                                            TC[:], op=ALU.mult)

                    dps = psD.tile([128, 2 * BL], f32, tag="tiny")
                    for cp in range(2):
                        for k in range(2):
                            w2 = W["w2t0"] if k == 0 else W["w2t1"]
                            nc.tensor.matmul(
                                dps[:, BL * cp:BL * (cp + 1)],
                                w2[:, 128 * cp:128 * (cp + 1)],
                                hT[:, BL * k:BL * (k + 1)],
                                start=(k == 0), stop=(k == 1))
                    dT_sb = dsb.tile([128, 2 * BL], f32, tag="dT")
                    nc.vector.tensor_copy(dT_sb[:], dps[:])

                    for g in range(NG):
                        b0 = g * GB
                        Sg = psS.tile([128, JJ * GB], f32, tag=f"S{g}")
                        for bi in range(GB):
                            b = b0 + bi
                            Tp0 = dtp.tile([128, Li], f32, tag="tprime")
                            nc.scalar.activation(
                                Tp0[:], encproj[0][:, Li * b:Li * (b + 1)],
                                AF.Tanh, bias=dT_sb[:, b:b + 1])
                            Tp1 = dtp.tile([128, Li], f32, tag="tprime")
                            nc.scalar.activation(
                                Tp1[:], encproj[1][:, Li * b:Li * (b + 1)],
                                AF.Tanh, bias=dT_sb[:, BL + b:BL + b + 1])
                            for j in range(JJ):
                                nc.tensor.matmul(
                                    Sg[:, JJ * bi + j:JJ * bi + j + 1],
                                    Tp0[:, 128 * j:128 * (j + 1)],
                                    W["vcol"][:, 0:1],
                                    start=True, stop=False)
                                nc.tensor.matmul(
                                    Sg[:, JJ * bi + j:JJ * bi + j + 1],
                                    Tp1[:, 128 * j:128 * (j + 1)],
                                    W["vcol"][:, 1:2],
                                    start=False, stop=True)
                        S_sb = dsb.tile([128, JJ * GB], f32, tag="S_sb")
                        nc.vector.tensor_copy(S_sb[:], Sg[:])
                        St_ps = psT.tile([JJ * GB, 128], f32, tag="StxT")
                        nc.tensor.transpose(St_ps[:], S_sb[:], W["ident"][:])
                        St = dsb.tile([JJ * GB, 128], f32, tag="St")
                        nc.vector.tensor_copy(St[:], St_ps[:])
                        nc.sync.dma_start(
                            stage2[t, JJ * GB * g:JJ * GB * (g + 1), :],
                            St[:])
                        mv8 = dsb.tile([JJ * GB, 8], f32, tag="mv8")
                        mi8 = dsb.tile([JJ * GB, 8], u32, tag="mi8")
                        nc.vector.max(mv8[:], St[:])
                        nc.vector.max_index(mi8[:], mv8[:], St[:])
                        mvlc = dsb.tile([JJ * GB, 2], f32, tag="mvlc")
                        nc.vector.tensor_copy(mvlc[:, 0:1], mv8[:, 0:1])
                        mif = dsb.tile([JJ * GB, 1], f32, tag="mif")
                        nc.vector.tensor_copy(mif[:], mi8[:, 0:1])
                        nc.vector.tensor_tensor(
                            mvlc[:, 1:2], mif[:],
                            W["jconst"][0:JJ * GB, :], op=ALU.add)
                        r0_ps = psD.tile([1, JJ * GB], f32, tag="tiny")
                        nc.tensor.transpose(r0_ps[:], mvlc[:, 0:1],
                                            W["ident"][0:JJ * GB, 0:JJ * GB])
                        r1_ps = psD.tile([1, JJ * GB], f32, tag="tiny")
                        nc.tensor.transpose(r1_ps[:], mvlc[:, 1:2],
                                            W["ident"][0:JJ * GB, 0:JJ * GB])
                        mvT = dsb.tile([1, JJ * GB], f32, tag="mvT")
                        nc.vector.tensor_copy(mvT[:], r0_ps[:])
                        lcT = dsb.tile([1, JJ * GB], f32, tag="lcT")
                        nc.vector.tensor_copy(lcT[:], r1_ps[:])
                        gmax = dsb.tile([1, GB], f32, tag="gmax")
                        nc.vector.tensor_reduce(
                            gmax[:],
                            mvT[0:1, :].rearrange("p (b j) -> p b j", j=JJ),
                            axis=AX.X, op=ALU.max)
                        eq = dsb.tile([1, JJ * GB], f32, tag="eq")
                        nc.vector.tensor_tensor(
                            eq[0:1, :].rearrange("p (b j) -> p b j", j=JJ),
                            mvT[0:1, :].rearrange("p (b j) -> p b j", j=JJ),
                            gmax[0:1, :].rearrange("p (b j) -> p b j", j=1)
                                .broadcast_to([1, GB, JJ]),
                            op=ALU.is_equal)
                        cand = dsb.tile([1, JJ * GB], f32, tag="cand")
                        nc.vector.tensor_scalar(cand[:], lcT[:],
                                                -9999.0, None, op0=ALU.add)
                        nc.vector.tensor_tensor(cand[:], cand[:], eq[:],
                                                op=ALU.mult)
                        nc.vector.tensor_scalar(cand[:], cand[:], 9999.0,
                                                None, op0=ALU.add)
                        lstar = dsb.tile([1, GB], f32, tag="lstar")
                        nc.vector.tensor_reduce(
                            lstar[:],
                            cand[0:1, :].rearrange("p (b j) -> p b j", j=JJ),
                            axis=AX.X, op=ALU.min)
                        nc.vector.tensor_copy(
                            ptr_row[0:1, BL * t + b0:BL * t + b0 + GB],
                            lstar[:])

                        if t == Lo - 1:
                            continue
                        colst = dsb.tile([1, GB], f32, tag="colst")
                        nc.vector.tensor_tensor(
                            colst[:], lstar[:],
                            W["bconst"][0:1, b0:b0 + GB], op=ALU.add)
                        c_ps = psD.tile([GB, 1], f32, tag="tiny")
                        nc.tensor.transpose(c_ps[:], colst[:],
                                            W["ident"][0:1, 0:1])
                        c16 = dsb.tile([GB, 1], f32, tag="c16")
                        nc.vector.tensor_copy(c16[:], c_ps[:])
                        i_ps = psD.tile([128, 1], f32, tag="tiny")
                        nc.tensor.matmul(i_ps[:], W["selm"][:], c16[:],
                                         start=True, stop=True)
                        idx16 = dsb.tile([128, 1], i16, tag="idx16")
                        nc.vector.tensor_copy(idx16[:], i_ps[:])
                        gat = dsb.tile([128, 1, 128], f32, tag=f"gat{g}")
                        gcnt[0] += 1
                        nc.gpsimd.dma_gather(
                            gat[:], emb_rows[:, :], idx16[:],
                            num_idxs=GB, num_idxs_reg=GB,
                            elem_size=E).then_inc(gsem, 16)
                        with tc.tile_critical():
                            nc.tensor.wait_ge(gsem, 16 * gcnt[0])
                            x_ps = psT.tile([E, GB], f32, tag="StxT")
                            nc.tensor.transpose(
                                x_ps[:], gat[0:GB, 0, :],
                                W["ident"][0:GB, 0:GB])
                        if g == 0:
                            xT = dst.tile([E, BL], f32, tag="xTd")
                        nc.vector.tensor_copy(xT[:, b0:b0 + GB], x_ps[:])

            # ---- pointers epilogue ----
            with tc.tile_pool(name="pe_sb", bufs=1) as psb2, \
                 tc.tile_pool(name="pe_ps", bufs=1, space="PSUM") as peps:
                pt2 = psb2.tile([Lo, BL], f32, tag="pt2")
                nc.sync.dma_start(pt2[:], ptr_row[:])
                pt_ps = peps.tile([BL, Lo], f32, tag="ptps")
                nc.tensor.transpose(pt_ps[:], pt2[:],
                                    W["ident"][0:Lo, 0:Lo])
                pt_i = psb2.tile([BL, Lo], i32, tag="pti")
                nc.vector.tensor_copy(pt_i[:], pt_ps[:])
                nc.sync.dma_start(ptr_o[:], pt_i[:])

            # ================= LOGITS EPILOGUE =================
            with tc.tile_pool(name="ep_sb", bufs=3) as xsb, \
                 tc.tile_pool(name="ep_ps", bufs=2, space="PSUM") as xps:
                for b in range(BL):
                    tin = xsb.tile([Lo, JJ * 128], f32, tag="tin")
                    nc.sync.dma_start(
                        tin[:].rearrange("t (j p) -> t j p", j=JJ),
                        stage2[:, JJ * b:JJ * (b + 1), :])
                    o_ps = xps.tile([128, JJ * Lo], f32, tag="ops")
                    for j in range(JJ):
                        nc.tensor.transpose(
                            o_ps[:, Lo * j:Lo * (j + 1)],
                            tin[:, 128 * j:128 * (j + 1)],
                            W["ident"][0:Lo, 0:Lo])
                    osb = xsb.tile([128, JJ * Lo], f32, tag="osb")
                    nc.vector.tensor_copy(osb[:], o_ps[:])
                    nc.sync.dma_start(
                        logits_o.rearrange(
                            "b (j p) t -> b p j t", j=JJ)[b, :, :, :],
                        osb[:].rearrange("p (j t) -> p j t", j=JJ))

    nc.compile()
    return nc


def host_prep(inputs, emb_W, emb_b, enc_Wih, enc_Whh, enc_b,
              dec_Wih, dec_Whh, dec_b, att_W1, att_W2, att_V, dec_input0,
              Li, Lo, ncores=NCORES):
    ROWS = BL * Li
    shared = {
        "wih_e": np.ascontiguousarray(_perm_rows(enc_Wih).T),
        "whh_e": np.ascontiguousarray(_perm_rows(enc_Whh).T),
        "wih_d": np.ascontiguousarray(_perm_rows(dec_Wih).T),
        "whh_d": np.ascontiguousarray(_perm_rows(dec_Whh).T),
        "w1t": np.ascontiguousarray(att_W1.T),
        "w2t": np.ascontiguousarray(att_W2.T),
        "vcol": np.ascontiguousarray(att_V[0].reshape(2, 128).T),
        "embw": np.ascontiguousarray(np.vstack([emb_W.T, emb_b[None, :]])),
        "x0t": np.ascontiguousarray(np.tile(dec_input0[:, None], (1, BL))),
        "selm": np.ascontiguousarray(
            (np.arange(GB)[:, None] == (np.arange(128)[None, :] % 16))
            .astype(np.float32)),
        "ident": np.eye(128, dtype=np.float32),
        "jconst": (128.0 * (np.arange(128) % max(Li // 128, 1))).astype(
            np.float32).reshape(128, 1),
        "bconst": (float(Li) * np.arange(BL)).astype(
            np.float32).reshape(1, BL),
    }
    if np.any(enc_b):
        eb = _perm_rows(enc_b[:, None].astype(np.float32)).reshape(8, 128)
        shared["encb"] = np.ascontiguousarray(
            np.repeat(eb[:, :, None], BL, axis=2)
            .transpose(1, 0, 2).reshape(128, 8 * BL))
    if np.any(dec_b):
        db = _perm_rows(dec_b[:, None].astype(np.float32)).reshape(8, 128)
        shared["decb"] = np.ascontiguousarray(
            np.repeat(db[:, :, None], BL, axis=2)
            .transpose(1, 0, 2).reshape(128, 8 * BL))

    in_maps = []
    for i in range(ncores):
        sl = inputs[BL * i:BL * (i + 1), :Li, :]
        inT = sl.transpose(2, 0, 1).reshape(C, ROWS)
        inT = np.vstack([inT, np.ones((1, ROWS), np.float32)])
        m = dict(shared)
        m["inT"] = np.ascontiguousarray(inT)
        in_maps.append(m)
    return in_maps


class Runner:
    """Reusable PJRT runner: traces/jits the bass program once, then every
    call only pays transfers + device execution (replicates
    bass2jax.run_bass_via_pjrt but caches the jitted sharded callable)."""

    def __init__(self, nc, n_cores):
        import jax
        from jax.sharding import Mesh, PartitionSpec, NamedSharding
        from jax.experimental.shard_map import shard_map
        from concourse import bass2jax
        from concourse import mybir as mb
        bass2jax.install_neuronx_cc_hook()
        self.jax = jax
        self.n_cores = n_cores
        pname = (nc.partition_id_tensor.name
                 if nc.partition_id_tensor else None)
        in_names, out_names, out_avals = [], [], []
        for alloc in nc.m.functions[0].allocations:
            if not isinstance(alloc, mb.MemoryLocationSet):
                continue
            name = alloc.memorylocations[0].name
            if alloc.kind == "ExternalInput":
                if name != pname:
                    in_names.append(name)
            elif alloc.kind == "ExternalOutput":
                out_avals.append(jax.core.ShapedArray(
                    tuple(alloc.tensor_shape), mb.dt.np(alloc.dtype)))
                out_names.append(name)
        self.n_params = len(in_names)
        self.out_names = out_names
        self.out_avals = out_avals
        all_in = list(in_names) + list(out_names)
        if pname is not None:
            all_in.append(pname)
        self.in_names = in_names

        def _body(*args):
            operands = list(args)
            if pname is not None:
                operands.append(bass2jax.partition_id_tensor())
            return tuple(bass2jax._bass_exec_p.bind(
                *operands,
                out_avals=tuple(out_avals),
                in_names=tuple(all_in),
                out_names=tuple(out_names),
                lowering_input_output_aliases=(),
                sim_require_finite=True,
                sim_require_nnan=True,
                nc=nc,
            ))

        devices = jax.devices()[:n_cores]
        self.mesh = Mesh(np.asarray(devices), ("core",))
        self.sharding = NamedSharding(self.mesh, PartitionSpec("core"))
        n_outs = len(out_names)
        donate = tuple(range(self.n_params, self.n_params + n_outs))
        self.sharded = jax.jit(
            shard_map(_body, mesh=self.mesh,
                      in_specs=(PartitionSpec("core"),) * (self.n_params + n_outs),
                      out_specs=(PartitionSpec("core"),) * n_outs,
                      check_rep=False),
            donate_argnums=donate, keep_unused=True)

    def place_inputs(self, in_maps):
        concat = [np.concatenate([np.asarray(m[n]) for m in in_maps], axis=0)
                  for n in self.in_names]
        return [self.jax.device_put(a, self.sharding) for a in concat]

    def make_zeros(self):
        jnp = self.jax.numpy
        outs = []
        for av in self.out_avals:
            z = self.jax.device_put(
                jnp.zeros((self.n_cores * av.shape[0], *av.shape[1:]),
                          av.dtype), self.sharding)
            outs.append(z)
        return outs

    def run_placed(self, dev_in):
        out_arrs = self.sharded(*dev_in, *self.make_zeros())
        for a in out_arrs:
            a.block_until_ready()
        return out_arrs

    def __call__(self, in_maps):
        dev_in = self.place_inputs(in_maps)
        out_arrs = self.run_placed(dev_in)
        res = []
        for c in range(self.n_cores):
            d = {}
            for i, name in enumerate(self.out_names):
                d[name] = np.asarray(out_arrs[i]).reshape(
                    self.n_cores, *self.out_avals[i].shape)[c]
            res.append(d)
        return res


_CACHE = {}


def get_runner(Li=LI_FULL, Lo=LO_FULL, nz_e=False, nz_d=False):
    key = (Li, Lo, nz_e, nz_d)
    if key not in _CACHE:
        nc = build_program(Li, Lo, nz_e, nz_d)
        _CACHE[key] = Runner(nc, NCORES)
    return _CACHE[key]


def kernel(inputs, emb_W, emb_b, enc_Wih, enc_Whh, enc_b,
           dec_Wih, dec_Whh, dec_b, att_W1, att_W2, att_V, dec_input0):
    args = [np.asarray(a, np.float32) for a in
            (inputs, emb_W, emb_b, enc_Wih, enc_Whh, enc_b,
             dec_Wih, dec_Whh, dec_b, att_W1, att_W2, att_V, dec_input0)]
    nz_e = bool(np.any(args[5]))
    nz_d = bool(np.any(args[8]))
    runner = get_runner(LI_FULL, LO_FULL, nz_e, nz_d)
    in_maps = host_prep(*args, Li=LI_FULL, Lo=LO_FULL)
    res = runner(in_maps)
    logits = np.concatenate([r["logits"] for r in res], axis=0)
    pointers = np.concatenate([r["pointers"] for r in res], axis=0)
    return logits, pointers.astype(np.int32)


# revision 11
# speedup vs baseline: 1.0379x; 1.0379x over previous
"""PointerNet Trainium2 Bass kernel.

kernel(**inputs) takes full unsharded numpy inputs (setup_inputs()) and
returns (logits, pointers) matching reference(). Batch B=256 is sharded
across 8 NeuronCores (32 rows each) with replicated weights; one SPMD
Bass program computes the whole model on-chip.

Per-core layout notes (col = b*Li + l everywhere):
  - enc_proj^T kept SBUF-resident as two [128, 32*Li] chunk tiles.
  - decoder attention: tanh fused with per-(b,chunk) bias on ACT;
    V-reduction via stationary-lhsT matmuls (T' tile as weights, V col
    streams) accumulating scores directly in PSUM in argmax layout.
  - two-stage argmax: DVE max/max_index per transposed-score row, then
    PE transpose + compare/select for the cross-partition reduction with
    exact first-occurrence tie semantics (matches jnp.argmax).
  - pointer gather: gpsimd dma_gather from a precomputed embedding-row
    table in DRAM (idx replication via a small selector matmul).
  - batch processed in two groups of 16 so each group's serial
    argmax/gather/LSTM chain hides under the other group's ACT stream.
"""
import sys
sys.path.insert(0, "/opt/trn_rl_repo")
import numpy as np
import concourse.bass as bass
import concourse.bacc as bacc
import concourse.tile as tile
from concourse import mybir
from concourse.bass_utils import run_bass_kernel_spmd

f32 = mybir.dt.float32
i32 = mybir.dt.int32
i16 = mybir.dt.int16
u32 = mybir.dt.uint32
AF = mybir.ActivationFunctionType
ALU = mybir.AluOpType
AX = mybir.AxisListType

B, C, E, H = 256, 16, 128, 256
LI_FULL, LO_FULL = 512, 128
NCORES = 8
BL = B // NCORES          # 32 local batch
NG = 2                    # batch groups
GB = BL // NG             # 16 per group
PERM = [0, 1, 2, 3, 6, 7, 4, 5]   # gate chunks (i,i,f,f,o,o,g,g)


def _perm_rows(w):
    return np.concatenate([w[128 * c:128 * (c + 1)] for c in PERM], axis=0)


def build_program(Li, Lo, nonzero_enc_b, nonzero_dec_b, num_devices=NCORES):
    ROWS = BL * Li
    JJ = Li // 128
    nc = bacc.Bacc("TRN2", target_bir_lowering=False, debug=False,
                   num_devices=num_devices)

    def din(name, shape, dt=f32):
        return nc.dram_tensor(name, list(shape), dt, kind="ExternalInput").ap()

    inT = din("inT", [C + 1, ROWS])
    wih_e = din("wih_e", [E, 4 * H])
    whh_e = din("whh_e", [H, 4 * H])
    wih_d = din("wih_d", [E, 4 * H])
    whh_d = din("whh_d", [H, 4 * H])
    w1t = din("w1t", [H, H])
    w2t = din("w2t", [H, H])
    vcol = din("vcol", [128, 2])
    embw = din("embw", [C + 1, E])
    x0t = din("x0t", [E, BL])
    selm = din("selm", [GB, 128])
    ident = din("ident", [128, 128])
    jconst = din("jconst", [128, 1])
    bconst = din("bconst", [1, BL])
    if nonzero_enc_b:
        encb = din("encb", [128, 8 * BL])
    if nonzero_dec_b:
        decb = din("decb", [128, 8 * BL])

    emb_rows = nc.dram_tensor("emb_rows", [ROWS, E], f32).ap()
    stage2 = nc.dram_tensor("stage2", [Lo, NG * JJ * GB, 128], f32).ap()

    logits_o = nc.dram_tensor("logits", [BL, Li, Lo], f32,
                              kind="ExternalOutput").ap()
    ptr_o = nc.dram_tensor("pointers", [BL, Lo], i32,
                           kind="ExternalOutput").ap()

    gsem = nc.alloc_semaphore("gsem")
    gcnt = [0]

    with tile.TileContext(nc) as tc:
        with tc.tile_pool(name="wpool", bufs=1) as wp:
            W = {}
            def load(name, ap, shape, dt=f32):
                t = wp.tile(list(shape), dt, tag=name)
                nc.sync.dma_start(t[:], ap)
                W[name] = t
            load("wih_e", wih_e[:], [E, 4 * H])
            load("whh_e0", whh_e[0:128, :], [128, 4 * H])
            load("whh_e1", whh_e[128:256, :], [128, 4 * H])
            load("wih_d", wih_d[:], [E, 4 * H])
            load("whh_d0", whh_d[0:128, :], [128, 4 * H])
            load("whh_d1", whh_d[128:256, :], [128, 4 * H])
            load("w1t0", w1t[0:128, :], [128, H])
            load("w1t1", w1t[128:256, :], [128, H])
            load("w2t0", w2t[0:128, :], [128, H])
            load("w2t1", w2t[128:256, :], [128, H])
            load("vcol", vcol[:], [128, 2])
            load("embw", embw[:], [C + 1, E])
            load("x0t", x0t[:], [E, BL])
            load("selm", selm[:], [GB, 128])
            load("ident", ident[:], [128, 128])
            load("jconst", jconst[:], [128, 1])
            load("bconst", bconst[:], [1, BL])
            if nonzero_enc_b:
                load("encb", encb[:], [128, 8 * BL])
            if nonzero_dec_b:
                load("decb", decb[:], [128, 8 * BL])

            encproj = [wp.tile([128, ROWS], f32, tag=f"encproj{c}",
                               name=f"encproj{c}")
                       for c in range(2)]
            ptr_row = wp.tile([1, BL * Lo], f32, tag="ptrrow")
            h_fin = wp.tile([128, 2 * BL], f32, tag="h_fin")
            c_fin = wp.tile([128, 2 * BL], f32, tag="c_fin")

            # ================= PROLOGUE: emb_rows table =================
            with tc.tile_pool(name="pro_sb", bufs=3) as psb, \
                 tc.tile_pool(name="pro_ps", bufs=3, space="PSUM") as pps:
                for ch in range(ROWS // 128):
                    itc = psb.tile([C + 1, 128], f32, tag="itc")
                    nc.sync.dma_start(itc[:], inT[:, 128 * ch:128 * (ch + 1)])
                    er_ps = pps.tile([128, E], f32, tag="erps")
                    nc.tensor.matmul(er_ps[:], itc[:], W["embw"][:],
                                     start=True, stop=True)
                    er_sb = psb.tile([128, E], f32, tag="ersb")
                    nc.vector.tensor_copy(er_sb[:], er_ps[:])
                    nc.sync.dma_start(
                        emb_rows[128 * ch:128 * (ch + 1), :], er_sb[:])

            # ================= ENCODER =================
            with tc.tile_pool(name="enc_sb", bufs=3) as esb, \
                 tc.tile_pool(name="enc_st", bufs=2) as est, \
                 tc.tile_pool(name="enc_ps", bufs=2, space="PSUM") as eps:
                hT = est.tile([128, 2 * BL], f32, tag="hT")
                cT = est.tile([128, 2 * BL], f32, tag="cT")
                nc.vector.memset(hT[:], 0.0)
                nc.vector.memset(cT[:], 0.0)

                SLAB = 64
                slab = None
                for t in range(Li):
                    if t % SLAB == 0:
                        slab = esb.tile([C + 1, SLAB * BL], f32, tag="slab")
                        nc.sync.dma_start(
                            slab[:],
                            inT[:, BL * t:BL * (t + SLAB)])
                    xs = slab[:, BL * (t % SLAB):BL * (t % SLAB + 1)]
                    xcol_ps = eps.tile([E, BL], f32, tag="xcol")
                    nc.tensor.matmul(xcol_ps[:], W["embw"][:], xs[:],
                                     start=True, stop=True)
                    xcol = esb.tile([E, BL], f32, tag="xcolsb")
                    nc.vector.tensor_copy(xcol[:], xcol_ps[:])

                    G = eps.tile([128, 8 * BL], f32, tag="G")
                    for cg in range(8):
                        nc.tensor.matmul(
                            G[:, BL * cg:BL * (cg + 1)],
                            W["wih_e"][:, 128 * cg:128 * (cg + 1)],
                            xcol[:], start=True, stop=False)
                        for k in range(2):
                            whh = W["whh_e0"] if k == 0 else W["whh_e1"]
                            nc.tensor.matmul(
                                G[:, BL * cg:BL * (cg + 1)],
                                whh[:, 128 * cg:128 * (cg + 1)],
                                hT[:, BL * k:BL * (k + 1)],
                                start=False, stop=(k == 1))
                    if nonzero_enc_b:
                        nc.vector.tensor_tensor(G[:], G[:], W["encb"][:],
                                                op=ALU.add)
                    SG = esb.tile([128, 6 * BL], f32, tag="SG")
                    nc.scalar.activation(SG[:], G[:, 0:6 * BL], AF.Sigmoid)
                    TG = esb.tile([128, 2 * BL], f32, tag="TG")
                    nc.scalar.activation(TG[:], G[:, 6 * BL:8 * BL], AF.Tanh)
                    m1 = esb.tile([128, 2 * BL], f32, tag="m1")
                    nc.vector.tensor_tensor(m1[:], SG[:, 0:2 * BL], TG[:],
                                            op=ALU.mult)
                    m2 = esb.tile([128, 2 * BL], f32, tag="m2")
                    nc.vector.tensor_tensor(m2[:], SG[:, 2 * BL:4 * BL],
                                            cT[:], op=ALU.mult)
                    cT = est.tile([128, 2 * BL], f32, tag="cT")
                    nc.vector.tensor_tensor(cT[:], m1[:], m2[:], op=ALU.add)
                    TC = esb.tile([128, 2 * BL], f32, tag="TC")
                    nc.scalar.activation(TC[:], cT[:], AF.Tanh)
                    hT = est.tile([128, 2 * BL], f32, tag="hT")
                    nc.vector.tensor_tensor(hT[:], SG[:, 4 * BL:6 * BL],
                                            TC[:], op=ALU.mult)

                    pcol = eps.tile([128, 2 * BL], f32, tag="pcol")
                    for cp in range(2):
                        for k in range(2):
                            w1 = W["w1t0"] if k == 0 else W["w1t1"]
                            nc.tensor.matmul(
                                pcol[:, BL * cp:BL * (cp + 1)],
                                w1[:, 128 * cp:128 * (cp + 1)],
                                hT[:, BL * k:BL * (k + 1)],
                                start=(k == 0), stop=(k == 1))
                    for cp in range(2):
                        nc.vector.tensor_copy(
                            encproj[cp].rearrange(
                                "h (b l) -> h l b", l=Li)[:, t, :],
                            pcol[:, BL * cp:BL * (cp + 1)])

                nc.vector.tensor_copy(h_fin[:], hT[:])
                nc.vector.tensor_copy(c_fin[:], cT[:])

            # ================= DECODER =================
            with tc.tile_pool(name="dec_sb", bufs=3) as dsb, \
                 tc.tile_pool(name="dec_tp", bufs=4) as dtp, \
                 tc.tile_pool(name="dec_st", bufs=2) as dst, \
                 tc.tile_pool(name="psG", bufs=2, space="PSUM") as psG, \
                 tc.tile_pool(name="psS", bufs=1, space="PSUM") as psS, \
                 tc.tile_pool(name="psT", bufs=2, space="PSUM") as psT, \
                 tc.tile_pool(name="psD", bufs=2, space="PSUM") as psD:

                hT = h_fin
                cT = c_fin
                xT = W["x0t"]

                for t in range(Lo):
                    G = psG.tile([128, 8 * BL], f32, tag="G2")
                    for cg in range(8):
                        for k in range(2):
                            whh = W["whh_d0"] if k == 0 else W["whh_d1"]
                            nc.tensor.matmul(
                                G[:, BL * cg:BL * (cg + 1)],
                                whh[:, 128 * cg:128 * (cg + 1)],
                                hT[:, BL * k:BL * (k + 1)],
                                start=(k == 0), stop=False)
                        nc.tensor.matmul(
                            G[:, BL * cg:BL * (cg + 1)],
                            W["wih_d"][:, 128 * cg:128 * (cg + 1)],
                            xT[:], start=False, stop=True)
                    if nonzero_dec_b:
                        nc.vector.tensor_tensor(G[:], G[:], W["decb"][:],
                                                op=ALU.add)
                    SG = dsb.tile([128, 6 * BL], f32, tag="SGd")
                    nc.scalar.activation(SG[:], G[:, 0:6 * BL], AF.Sigmoid)
                    TG = dsb.tile([128, 2 * BL], f32, tag="TGd")
                    nc.scalar.activation(TG[:], G[:, 6 * BL:8 * BL], AF.Tanh)
                    m1 = dsb.tile([128, 2 * BL], f32, tag="m1d")
                    nc.vector.tensor_tensor(m1[:], SG[:, 0:2 * BL], TG[:],
                                            op=ALU.mult)
                    m2 = dsb.tile([128, 2 * BL], f32, tag="m2d")
                    nc.vector.tensor_tensor(m2[:], SG[:, 2 * BL:4 * BL],
                                            cT[:], op=ALU.mult)
                    cT = dst.tile([128, 2 * BL], f32, tag="cTd")
                    nc.vector.tensor_tensor(cT[:], m1[:], m2[:], op=ALU.add)
                    TC = dsb.tile([128, 2 * BL], f32, tag="TCd")
                    nc.scalar.activation(TC[:], cT[:], AF.Tanh)
                    hT = dst.tile([128, 2 * BL], f32, tag="hTd")
                    nc.vector.tensor_tensor(hT[:], SG[:, 4 * BL:6 * BL],
                                            TC[:], op=ALU.mult)

                    dps = psD.tile([128, 2 * BL], f32, tag="tiny")
                    for cp in range(2):
                        for k in range(2):
                            w2 = W["w2t0"] if k == 0 else W["w2t1"]
                            nc.tensor.matmul(
                                dps[:, BL * cp:BL * (cp + 1)],
                                w2[:, 128 * cp:128 * (cp + 1)],
                                hT[:, BL * k:BL * (k + 1)],
                                start=(k == 0), stop=(k == 1))
                    dT_sb = dsb.tile([128, 2 * BL], f32, tag="dT")
                    nc.vector.tensor_copy(dT_sb[:], dps[:])

                    for g in range(NG):
                        b0 = g * GB
                        Sg = psS.tile([128, JJ * GB], f32, tag=f"S{g}")
                        for bi in range(GB):
                            b = b0 + bi
                            Tp0 = dtp.tile([128, Li], f32, tag="tprime")
                            nc.scalar.activation(
                                Tp0[:], encproj[0][:, Li * b:Li * (b + 1)],
                                AF.Tanh, bias=dT_sb[:, b:b + 1])
                            Tp1 = dtp.tile([128, Li], f32, tag="tprime")
                            nc.scalar.activation(
                                Tp1[:], encproj[1][:, Li * b:Li * (b + 1)],
                                AF.Tanh, bias=dT_sb[:, BL + b:BL + b + 1])
                            for j in range(JJ):
                                nc.tensor.matmul(
                                    Sg[:, JJ * bi + j:JJ * bi + j + 1],
                                    Tp0[:, 128 * j:128 * (j + 1)],
                                    W["vcol"][:, 0:1],
                                    start=True, stop=False)
                                nc.tensor.matmul(
                                    Sg[:, JJ * bi + j:JJ * bi + j + 1],
                                    Tp1[:, 128 * j:128 * (j + 1)],
                                    W["vcol"][:, 1:2],
                                    start=False, stop=True)
                        S_sb = dsb.tile([128, JJ * GB], f32, tag="S_sb")
                        nc.vector.tensor_copy(S_sb[:], Sg[:])
                        St_ps = psT.tile([JJ * GB, 128], f32, tag="StxT")
                        nc.tensor.transpose(St_ps[:], S_sb[:], W["ident"][:])
                        St = dsb.tile([JJ * GB, 128], f32, tag="St")
                        nc.vector.tensor_copy(St[:], St_ps[:])
                        nc.sync.dma_start(
                            stage2[t, JJ * GB * g:JJ * GB * (g + 1), :],
                            St[:])
                        mv8 = dsb.tile([JJ * GB, 8], f32, tag="mv8")
                        mi8 = dsb.tile([JJ * GB, 8], u32, tag="mi8")
                        nc.vector.max(mv8[:], St[:])
                        nc.vector.max_index(mi8[:], mv8[:], St[:])
                        mvlc = dsb.tile([JJ * GB, 2], f32, tag="mvlc")
                        nc.vector.tensor_copy(mvlc[:, 0:1], mv8[:, 0:1])
                        mif = dsb.tile([JJ * GB, 1], f32, tag="mif")
                        nc.vector.tensor_copy(mif[:], mi8[:, 0:1])
                        nc.vector.tensor_tensor(
                            mvlc[:, 1:2], mif[:],
                            W["jconst"][0:JJ * GB, :], op=ALU.add)
                        r0_ps = psD.tile([1, JJ * GB], f32, tag="tiny")
                        nc.tensor.transpose(r0_ps[:], mvlc[:, 0:1],
                                            W["ident"][0:JJ * GB, 0:JJ * GB])
                        r1_ps = psD.tile([1, JJ * GB], f32, tag="tiny")
                        nc.tensor.transpose(r1_ps[:], mvlc[:, 1:2],
                                            W["ident"][0:JJ * GB, 0:JJ * GB])
                        mvT = dsb.tile([1, JJ * GB], f32, tag="mvT")
                        nc.vector.tensor_copy(mvT[:], r0_ps[:])
                        lcT = dsb.tile([1, JJ * GB], f32, tag="lcT")
                        nc.vector.tensor_copy(lcT[:], r1_ps[:])
                        gmax = dsb.tile([1, GB], f32, tag="gmax")
                        nc.vector.tensor_reduce(
                            gmax[:],
                            mvT[0:1, :].rearrange("p (b j) -> p b j", j=JJ),
                            axis=AX.X, op=ALU.max)
                        eq = dsb.tile([1, JJ * GB], f32, tag="eq")
                        nc.vector.tensor_tensor(
                            eq[0:1, :].rearrange("p (b j) -> p b j", j=JJ),
                            mvT[0:1, :].rearrange("p (b j) -> p b j", j=JJ),
                            gmax[0:1, :].rearrange("p (b j) -> p b j", j=1)
                                .broadcast_to([1, GB, JJ]),
                            op=ALU.is_equal)
                        cand = dsb.tile([1, JJ * GB], f32, tag="cand")
                        nc.vector.tensor_scalar(cand[:], lcT[:],
                                                -9999.0, None, op0=ALU.add)
                        nc.vector.tensor_tensor(cand[:], cand[:], eq[:],
                                                op=ALU.mult)
                        nc.vector.tensor_scalar(cand[:], cand[:], 9999.0,
                                                None, op0=ALU.add)
                        lstar = dsb.tile([1, GB], f32, tag="lstar")
                        nc.vector.tensor_reduce(
                            lstar[:],
                            cand[0:1, :].rearrange("p (b j) -> p b j", j=JJ),
                            axis=AX.X, op=ALU.min)
                        nc.vector.tensor_copy(
                            ptr_row[0:1, BL * t + b0:BL * t + b0 + GB],
                            lstar[:])

                        if t == Lo - 1:
                            continue
                        colst = dsb.tile([1, GB], f32, tag="colst")
                        nc.vector.scalar_tensor_tensor(
                            colst[:], lstar[:], float(BL),
                            W["bconst"][0:1, b0:b0 + GB],
                            op0=ALU.mult, op1=ALU.add)
                        c_ps = psD.tile([GB, 1], f32, tag="tiny")
                        nc.tensor.transpose(c_ps[:], colst[:],
                                            W["ident"][0:1, 0:1])
                        c16 = dsb.tile([GB, 1], f32, tag="c16")
                        nc.vector.tensor_copy(c16[:], c_ps[:])
                        i_ps = psD.tile([128, 1], f32, tag="tiny")
                        nc.tensor.matmul(i_ps[:], W["selm"][:], c16[:],
                                         start=True, stop=True)
                        idx16 = dsb.tile([128, 1], i16, tag="idx16")
                        nc.vector.tensor_copy(idx16[:], i_ps[:])
                        gat = dsb.tile([128, 1, 128], f32, tag=f"gat{g}")
                        gcnt[0] += 1
                        nc.gpsimd.dma_gather(
                            gat[:], emb_rows[:, :], idx16[:],
                            num_idxs=GB, num_idxs_reg=GB,
                            elem_size=E).then_inc(gsem, 16)
                        with tc.tile_critical():
                            nc.tensor.wait_ge(gsem, 16 * gcnt[0])
                            x_ps = psT.tile([E, GB], f32, tag="StxT")
                            nc.tensor.transpose(
                                x_ps[:], gat[0:GB, 0, :],
                                W["ident"][0:GB, 0:GB])
                        if g == 0:
                            xT = dst.tile([E, BL], f32, tag="xTd")
                        nc.vector.tensor_copy(xT[:, b0:b0 + GB], x_ps[:])

            # ---- pointers epilogue ----
            with tc.tile_pool(name="pe_sb", bufs=1) as psb2, \
                 tc.tile_pool(name="pe_ps", bufs=1, space="PSUM") as peps:
                pt2 = psb2.tile([Lo, BL], f32, tag="pt2")
                nc.sync.dma_start(pt2[:], ptr_row[:])
                pt_ps = peps.tile([BL, Lo], f32, tag="ptps")
                nc.tensor.transpose(pt_ps[:], pt2[:],
                                    W["ident"][0:Lo, 0:Lo])
                pt_i = psb2.tile([BL, Lo], i32, tag="pti")
                nc.vector.tensor_copy(pt_i[:], pt_ps[:])
                nc.sync.dma_start(ptr_o[:], pt_i[:])

            # ================= LOGITS EPILOGUE =================
            with tc.tile_pool(name="ep_sb", bufs=3) as xsb, \
                 tc.tile_pool(name="ep_ps", bufs=2, space="PSUM") as xps:
                for b in range(BL):
                    tin = xsb.tile([Lo, JJ * 128], f32, tag="tin")
                    nc.sync.dma_start(
                        tin[:].rearrange("t (j p) -> t j p", j=JJ),
                        stage2[:, JJ * b:JJ * (b + 1), :])
                    o_ps = xps.tile([128, JJ * Lo], f32, tag="ops")
                    for j in range(JJ):
                        nc.tensor.transpose(
                            o_ps[:, Lo * j:Lo * (j + 1)],
                            tin[:, 128 * j:128 * (j + 1)],
                            W["ident"][0:Lo, 0:Lo])
                    osb = xsb.tile([128, JJ * Lo], f32, tag="osb")
                    nc.vector.tensor_copy(osb[:], o_ps[:])
                    nc.sync.dma_start(
                        logits_o.rearrange(
                            "b (j p) t -> b p j t", j=JJ)[b, :, :, :],
                        osb[:].rearrange("p (j t) -> p j t", j=JJ))

    nc.compile()
    return nc


def host_prep(inputs, emb_W, emb_b, enc_Wih, enc_Whh, enc_b,
              dec_Wih, dec_Whh, dec_b, att_W1, att_W2, att_V, dec_input0,
              Li, Lo, ncores=NCORES):
    ROWS = BL * Li
    shared = {
        "wih_e": np.ascontiguousarray(_perm_rows(enc_Wih).T),
        "whh_e": np.ascontiguousarray(_perm_rows(enc_Whh).T),
        "wih_d": np.ascontiguousarray(_perm_rows(dec_Wih).T),
        "whh_d": np.ascontiguousarray(_perm_rows(dec_Whh).T),
        "w1t": np.ascontiguousarray(att_W1.T),
        "w2t": np.ascontiguousarray(att_W2.T),
        "vcol": np.ascontiguousarray(att_V[0].reshape(2, 128).T),
        "embw": np.ascontiguousarray(np.vstack([emb_W.T, emb_b[None, :]])),
        "x0t": np.ascontiguousarray(np.tile(dec_input0[:, None], (1, BL))),
        "selm": np.ascontiguousarray(
            (np.arange(GB)[:, None] == (np.arange(128)[None, :] % 16))
            .astype(np.float32)),
        "ident": np.eye(128, dtype=np.float32),
        "jconst": (128.0 * (np.arange(128) % max(Li // 128, 1))).astype(
            np.float32).reshape(128, 1),
        "bconst": np.arange(BL).astype(np.float32).reshape(1, BL),
    }
    if np.any(enc_b):
        eb = _perm_rows(enc_b[:, None].astype(np.float32)).reshape(8, 128)
        shared["encb"] = np.ascontiguousarray(
            np.repeat(eb[:, :, None], BL, axis=2)
            .transpose(1, 0, 2).reshape(128, 8 * BL))
    if np.any(dec_b):
        db = _perm_rows(dec_b[:, None].astype(np.float32)).reshape(8, 128)
        shared["decb"] = np.ascontiguousarray(
            np.repeat(db[:, :, None], BL, axis=2)
            .transpose(1, 0, 2).reshape(128, 8 * BL))

    in_maps = []
    for i in range(ncores):
        sl = inputs[BL * i:BL * (i + 1), :Li, :]
        inT = sl.transpose(2, 1, 0).reshape(C, ROWS)   # col = l*BL + b
        inT = np.vstack([inT, np.ones((1, ROWS), np.float32)])
        m = dict(shared)
        m["inT"] = np.ascontiguousarray(inT)
        in_maps.append(m)
    return in_maps


class Runner:
    """Reusable PJRT runner: traces/jits the bass program once, then every
    call only pays transfers + device execution (replicates
    bass2jax.run_bass_via_pjrt but caches the jitted sharded callable)."""

    def __init__(self, nc, n_cores):
        import jax
        from jax.sharding import Mesh, PartitionSpec, NamedSharding
        from jax.experimental.shard_map import shard_map
        from concourse import bass2jax
        from concourse import mybir as mb
        bass2jax.install_neuronx_cc_hook()
        self.jax = jax
        self.n_cores = n_cores
        pname = (nc.partition_id_tensor.name
                 if nc.partition_id_tensor else None)
        in_names, out_names, out_avals = [], [], []
        for alloc in nc.m.functions[0].allocations:
            if not isinstance(alloc, mb.MemoryLocationSet):
                continue
            name = alloc.memorylocations[0].name
            if alloc.kind == "ExternalInput":
                if name != pname:
                    in_names.append(name)
            elif alloc.kind == "ExternalOutput":
                out_avals.append(jax.core.ShapedArray(
                    tuple(alloc.tensor_shape), mb.dt.np(alloc.dtype)))
                out_names.append(name)
        self.n_params = len(in_names)
        self.out_names = out_names
        self.out_avals = out_avals
        all_in = list(in_names) + list(out_names)
        if pname is not None:
            all_in.append(pname)
        self.in_names = in_names

        def _body(*args):
            operands = list(args)
            if pname is not None:
                operands.append(bass2jax.partition_id_tensor())
            return tuple(bass2jax._bass_exec_p.bind(
                *operands,
                out_avals=tuple(out_avals),
                in_names=tuple(all_in),
                out_names=tuple(out_names),
                lowering_input_output_aliases=(),
                sim_require_finite=True,
                sim_require_nnan=True,
                nc=nc,
            ))

        devices = jax.devices()[:n_cores]
        self.mesh = Mesh(np.asarray(devices), ("core",))
        self.sharding = NamedSharding(self.mesh, PartitionSpec("core"))
        n_outs = len(out_names)
        donate = tuple(range(self.n_params, self.n_params + n_outs))
        self.sharded = jax.jit(
            shard_map(_body, mesh=self.mesh,
                      in_specs=(PartitionSpec("core"),) * (self.n_params + n_outs),
                      out_specs=(PartitionSpec("core"),) * n_outs,
                      check_rep=False),
            donate_argnums=donate, keep_unused=True)

    def place_inputs(self, in_maps):
        concat = [np.concatenate([np.asarray(m[n]) for m in in_maps], axis=0)
                  for n in self.in_names]
        return [self.jax.device_put(a, self.sharding) for a in concat]

    def make_zeros(self):
        jnp = self.jax.numpy
        outs = []
        for av in self.out_avals:
            z = self.jax.device_put(
                jnp.zeros((self.n_cores * av.shape[0], *av.shape[1:]),
                          av.dtype), self.sharding)
            outs.append(z)
        return outs

    def run_placed(self, dev_in):
        out_arrs = self.sharded(*dev_in, *self.make_zeros())
        for a in out_arrs:
            a.block_until_ready()
        return out_arrs

    def __call__(self, in_maps):
        dev_in = self.place_inputs(in_maps)
        out_arrs = self.run_placed(dev_in)
        res = []
        for c in range(self.n_cores):
            d = {}
            for i, name in enumerate(self.out_names):
                d[name] = np.asarray(out_arrs[i]).reshape(
                    self.n_cores, *self.out_avals[i].shape)[c]
            res.append(d)
        return res


_CACHE = {}


def get_runner(Li=LI_FULL, Lo=LO_FULL, nz_e=False, nz_d=False):
    key = (Li, Lo, nz_e, nz_d)
    if key not in _CACHE:
        nc = build_program(Li, Lo, nz_e, nz_d)
        _CACHE[key] = Runner(nc, NCORES)
    return _CACHE[key]


def kernel(inputs, emb_W, emb_b, enc_Wih, enc_Whh, enc_b,
           dec_Wih, dec_Whh, dec_b, att_W1, att_W2, att_V, dec_input0):
    args = [np.asarray(a, np.float32) for a in
            (inputs, emb_W, emb_b, enc_Wih, enc_Whh, enc_b,
             dec_Wih, dec_Whh, dec_b, att_W1, att_W2, att_V, dec_input0)]
    nz_e = bool(np.any(args[5]))
    nz_d = bool(np.any(args[8]))
    runner = get_runner(LI_FULL, LO_FULL, nz_e, nz_d)
    in_maps = host_prep(*args, Li=LI_FULL, Lo=LO_FULL)
    res = runner(in_maps)
    logits = np.concatenate([r["logits"] for r in res], axis=0)
    pointers = np.concatenate([r["pointers"] for r in res], axis=0)
    return logits, pointers.astype(np.int32)
